# revision 31
# baseline (speedup 1.0000x reference)
"""Trainium2 Bass kernel for PolyIoULoss (rotated-box IoU loss, mean reduction).

Same sort-free clipped-boundary algorithm as before, restructured for engine
balance and instruction cost:

- Shared inverses: all 8 edge slopes are {pw,ph,tw,th} x {cosf,sinf}, so only
  TWO guarded reciprocal chains (1/cosf, 1/sinf) are needed plus four exact
  reciprocals 1/w via ACT Exp(-Ln(w)) (w >= 2, always safe).  The old kernel
  ran 8 full chains (32 DVE ops); this runs 2 chains + 8 products.
- cosf = Sin(pi/2 - |phi|) on ACT (cos is even, |phi| < pi keeps Sin in its
  accurate range) - replaces the 3-op half-angle identity.
- Engine cost model (per [128,512] instruction): DVE tt bf16 ~327ns (2x mode),
  DVE tensor_scalar bf16 ~193ns (4x mode), DVE stt/f32 ~593ns; Pool
  TensorScalarPtr ~806ns (dtype-blind, 2 fused ALU ops); ACT ~612ns.
  Ops are placed to balance DVE/Pool/ACT; Pool single add/sub/mult go through
  scalar_tensor_tensor (806ns) instead of tensor_tensor (1111ns).

Sharding: 524288 pairs split as 65536 per core (8 cores), [128 x 512] tiles.
Each core returns per-partition sums of log(iou); host combines and negates.
"""

import numpy as np

import concourse.bacc as bacc
import concourse.tile as tile
from concourse import mybir
from concourse.mybir import AluOpType as Op, ActivationFunctionType as Fn
from concourse.bass_utils import run_bass_kernel_spmd

N_TOTAL = 524288
NCORES = 8
NPER = N_TOTAL // NCORES          # 65536 pairs per core
P = 128
F = NPER // P                     # 512 pairs per partition

HPI = float(np.pi / 2)
EPS = 1e-6                        # from the loss definition
F32 = mybir.dt.float32
BF16 = mybir.dt.bfloat16


class _Var:
    __slots__ = ("ap", "tag")

    def __init__(self, ap, tag):
        self.ap = ap
        self.tag = tag


def _ap(x):
    return x.ap[:, :] if isinstance(x, _Var) else x


class _Slots:
    """Manual slot allocator over Tile pool tags (see previous version)."""

    MAX_F32 = 24
    MAX_BF16 = 110

    def __init__(self, pool):
        self.pool = pool
        import collections
        self.free_tags = {F32: collections.deque(), BF16: collections.deque()}
        self.count = {F32: 0, BF16: 0}
        self.max = {F32: self.MAX_F32, BF16: self.MAX_BF16}
        self.pfx = {F32: "s", BF16: "h"}

    def tile(self, dt=F32):
        if self.count[dt] < self.max[dt]:
            tag = f"{self.pfx[dt]}{self.count[dt]}"
            self.count[dt] += 1
        else:
            tag = self.free_tags[dt].popleft()
        t = self.pool.tile([P, F], dt, tag=tag)
        return _Var(t, tag + "|" + str(dt))

    def free(self, *tiles):
        for t in tiles:
            tag, dts = t.tag.rsplit("|", 1)
            dt = F32 if dts == str(F32) else BF16
            self.free_tags[dt].append(tag)


def _build():
    nc = bacc.Bacc(None, target_bir_lowering=False)
    pred_h = nc.dram_tensor("pred", [NPER, 5], F32, kind="ExternalInput")
    tgt_h = nc.dram_tensor("target", [NPER, 5], F32, kind="ExternalInput")
    out_h = nc.dram_tensor("out", [P, 1], F32, kind="ExternalOutput")

    V = nc.vector
    G = nc.gpsimd
    A = nc.scalar

    with tile.TileContext(nc) as tc:
        with tc.tile_pool(name="io", bufs=1) as io_pool, \
             tc.tile_pool(name="consts", bufs=1) as cpool, \
             tc.tile_pool(name="main", bufs=1) as pool:
            s = _Slots(pool)

            _kcache = {}

            def konst(val):
                if val not in _kcache:
                    t = cpool.tile([P, 1], F32, tag=f"c{len(_kcache)}")
                    nc.gpsimd.memset(t[:, :], val)
                    _kcache[val] = t
                return _kcache[val][:, :]

            PRED = io_pool.tile([P, F, 5], F32, tag="PRED")
            TGT = io_pool.tile([P, F, 5], F32, tag="TGT")
            nc.sync.dma_start(out=PRED, in_=pred_h[:, :].rearrange("(p f) c -> p f c", p=P))
            nc.sync.dma_start(out=TGT, in_=tgt_h[:, :].rearrange("(p f) c -> p f c", p=P))

            pcx, pcy, pw, ph, pth = (PRED[:, :, j] for j in range(5))
            tcx, tcy, tw, th, tth = (TGT[:, :, j] for j in range(5))

            def tt(eng, x, y, op, dt=BF16):
                o = s.tile(dt)
                eng.tensor_tensor(_ap(o), _ap(x), _ap(y), op)
                return o

            def ts(eng, x, s1, op, s2=None, op2=None, dt=BF16, accum_out=None):
                o = s.tile(dt)
                if op2 is None:
                    eng.tensor_scalar(_ap(o), _ap(x), s1, s2, op,
                                      accum_out=accum_out)
                else:
                    eng.tensor_scalar(_ap(o), _ap(x), s1, s2, op, op2,
                                      accum_out=accum_out)
                return o

            def stt(eng, x, scalar, y, op0, op1, dt=BF16):
                # (x op0 scalar) op1 y
                o = s.tile(dt)
                eng.scalar_tensor_tensor(_ap(o), _ap(x), scalar, _ap(y), op0, op1)
                return o

            # Pool helpers (Pool ISA: tensor_tensor add/sub/mult + tensor_scalar)
            def gadd(x, y, dt=BF16):
                return tt(G, x, y, Op.add, dt=dt)

            def gsub(x, y, dt=BF16):       # x - y
                return tt(G, x, y, Op.subtract, dt=dt)

            def gmul(x, y, dt=BF16):
                return tt(G, x, y, Op.mult, dt=dt)

            def act(x, func, bias=0.0, scale=1.0, accum_out=None, dt=BF16):
                o = s.tile(dt)
                if isinstance(bias, float) and bias not in (0.0, 1.0):
                    bias = konst(bias)
                if isinstance(scale, float) and scale not in (0.0, 1.0):
                    scale = konst(scale)
                A.activation(out=_ap(o), in_=_ap(x),
                             func=func, bias=bias, scale=scale, accum_out=accum_out)
                return o

            # ---- Phase A: angles (ACT) + input-touch linear ops (Pool/DVE) ----
            phi = tt(V, tth, pth, Op.subtract, dt=F32)   # critical chain: DVE
            sinp = act(pth, Fn.Sin)
            cosp = act(pth, Fn.Sin, bias=HPI, scale=-1.0)
            sinf = act(phi, Fn.Sin)
            aphi = act(phi, Fn.Abs, dt=F32)
            cosf = act(aphi, Fn.Sin, bias=HPI, scale=-1.0)
            s.free(phi, aphi)

            Dx = tt(V, tcx, pcx, Op.subtract)            # f32 in -> bf16 (593)
            Dy = tt(V, tcy, pcy, Op.subtract)
            ar1 = gmul(pw, ph, dt=F32)                   # Pool, f32 for den
            ar2 = gmul(tw, th, dt=F32)
            a = ts(G, pw, 0.5, Op.mult)
            b = ts(G, ph, 0.5, Op.mult)
            a2 = ts(G, tw, 0.5, Op.mult)
            b2 = ts(G, th, 0.5, Op.mult)

            # square the trig terms now so all Sin/Square/Abs ACT work is
            # contiguous (one act-table load), then switch to Ln/Exp once.
            qc = act(cosf, Fn.Square)
            qs = act(sinf, Fn.Square)

            # exact reciprocals of box extents (>= 2.0, no guard needed)
            ipw = act(act(pw, Fn.Ln, dt=F32), Fn.Exp, scale=-1.0)
            iph = act(act(ph, Fn.Ln, dt=F32), Fn.Exp, scale=-1.0)
            itw = act(act(tw, Fn.Ln, dt=F32), Fn.Exp, scale=-1.0)
            ith = act(act(th, Fn.Ln, dt=F32), Fn.Exp, scale=-1.0)

            # rotate delta into pred frame
            t1 = tt(V, Dx, cosp, Op.mult)
            t2 = tt(V, Dy, sinp, Op.mult)
            t3 = tt(V, Dx, sinp, Op.mult)
            t4 = tt(V, Dy, cosp, Op.mult)
            dx = tt(V, t1, t2, Op.add)
            dy = tt(V, t4, t3, Op.subtract)
            s.free(t1, t2, t3, t4, Dx, Dy)

            ab = tt(V, a, b, Op.mult)
            ab2 = gmul(a2, b2)

            # ---- Phase B: target geometry in pred frame ------------------------
            A_ = tt(V, a2, cosf, Op.mult)
            B_ = tt(V, b2, sinf, Op.mult)
            C_ = tt(V, a2, sinf, Op.mult)
            D_ = tt(V, b2, cosf, Op.mult)

            m1 = gmul(dx, C_)
            m2 = gmul(dy, A_)
            uu = tt(V, m1, m2, Op.add)
            s.free(m1, m2)
            m3 = gmul(dx, D_)
            m4 = gmul(dy, B_)
            vv = tt(V, m3, m4, Op.subtract)
            s.free(m3, m4)

            S1 = gadd(A_, B_)
            S2 = gsub(A_, B_)
            S3 = gadd(C_, D_)
            S4 = gsub(C_, D_)
            s.free(A_, B_, C_, D_)
            g0x = tt(V, dx, S1, Op.subtract)
            g2x = tt(V, dx, S1, Op.add)
            g1x = tt(V, dx, S2, Op.add)
            g3x = tt(V, dx, S2, Op.subtract)
            s.free(S1, S2)
            g0y = gadd(dy, S4)
            g2y = gsub(dy, S4)
            g1y = gsub(dy, S3)
            g3y = gadd(dy, S3)
            s.free(S3, S4)

            # ---- Phase C: pred corners in target frame -------------------------
            e1 = tt(V, a, dx, Op.subtract)
            e2 = stt(V, dx, -1.0, a, Op.mult, Op.subtract)   # -dx - a
            f1 = tt(V, b, dy, Op.subtract)
            f2 = stt(V, dy, -1.0, b, Op.mult, Op.subtract)   # -dy - b
            s.free(dx, dy)

            e1c = tt(V, e1, cosf, Op.mult)
            e2c = tt(V, e2, cosf, Op.mult)
            f1s = tt(V, f1, sinf, Op.mult)
            f2s = tt(V, f2, sinf, Op.mult)
            e1s = tt(V, e1, sinf, Op.mult)
            e2s = tt(V, e2, sinf, Op.mult)
            f1c = tt(V, f1, cosf, Op.mult)
            f2c = tt(V, f2, cosf, Op.mult)
            s.free(e1, e2, f1, f2)

            xi0 = tt(V, e2c, f2s, Op.subtract)
            xi1 = tt(V, e1c, f2s, Op.subtract)
            xi2 = tt(V, e1c, f1s, Op.subtract)
            xi3 = tt(V, e2c, f1s, Op.subtract)
            et0 = tt(V, e2s, f2c, Op.add)
            et1 = tt(V, e1s, f2c, Op.add)
            et2 = tt(V, e1s, f1c, Op.add)
            et3 = tt(V, e2s, f1c, Op.add)
            s.free(e1c, e2c, f1s, f2s, e1s, e2s, f1c, f2c)

            # ---- Phase D: two guarded reciprocal chains ------------------------
            qcc = ts(V, qc, 1e-30, Op.max, dt=F32)   # f32: recip views int32 bits
            qsc = ts(V, qs, 1e-30, Op.max, dt=F32)
            s.free(qc, qs)
            rc = s.tile()
            V.reciprocal_approx_fast(out=_ap(rc), in_=_ap(qcc))
            rs = s.tile()
            V.reciprocal_approx_fast(out=_ap(rs), in_=_ap(qsc))
            s.free(qcc, qsc)
            ic = tt(V, cosf, rc, Op.mult)        # ~1/cosf (0 as cosf -> 0)
            is_ = tt(V, sinf, rs, Op.mult)       # ~1/sinf
            s.free(rc, rs)

            iu1 = tt(V, ipw, ic, Op.mult)
            iu2 = tt(V, ipw, is_, Op.mult)
            niw1 = tt(V, iph, is_, Op.mult)      # = -iw1 (s-signs flipped below)
            iw2 = tt(V, iph, ic, Op.mult)
            iA = tt(V, itw, ic, Op.mult)
            iC = tt(V, itw, is_, Op.mult)
            iB = tt(V, ith, is_, Op.mult)
            iD = tt(V, ith, ic, Op.mult)

            # ---- Phase E: alphas via ratios ------------------------------------
            ICF = stt(V, ic, -1.0, ic, Op.mult, Op.max)    # |ic| on DVE
            ISF = stt(V, is_, -1.0, is_, Op.mult, Op.max)
            s.free(ic, is_)
            R1 = tt(V, a2, ipw, Op.mult)
            R2 = tt(V, b2, ipw, Op.mult)
            R3 = tt(V, a2, iph, Op.mult)
            R4 = tt(V, b2, iph, Op.mult)
            R5 = gmul(a, itw)
            R6 = gmul(b, itw)
            R7 = gmul(a, ith)
            R8 = gmul(b, ith)
            s.free(ipw, iph, itw, ith, a, b, a2, b2)
            AXU = tt(V, R1, ICF, Op.mult)
            AEU = tt(V, R2, ISF, Op.mult)
            AXW = tt(V, R3, ISF, Op.mult)
            AEW = tt(V, R4, ICF, Op.mult)
            AXA = tt(V, R5, ICF, Op.mult)
            AYC = tt(V, R6, ISF, Op.mult)
            AXB = tt(V, R7, ISF, Op.mult)
            AYD = tt(V, R8, ICF, Op.mult)
            s.free(R1, R2, R3, R4, R5, R6, R7, R8, ICF, ISF)

            # ---- Phase F: per-edge clip intervals, software-pipelined ----------
            # (c1, i1, s1, Aa1, c2, i2, s2, Aa2) per edge; all b-products
            # first, then n (Pool) / h (DVE) in bulk, then the min cascade, so
            # Pool runs ahead and DVE never waits on the current edge.
            EDGES = [
                (xi0, iu1, +1, AXU, et0, iu2, +1, AEU),
                (xi1, niw1, -1, AXW, et1, iw2, +1, AEW),
                (xi2, iu1, -1, AXU, et2, iu2, -1, AEU),
                (xi3, niw1, +1, AXW, et3, iw2, -1, AEW),
                (g0x, iA, +1, AXA, g0y, iC, -1, AYC),
                (g1x, iB, +1, AXB, g1y, iD, +1, AYD),
                (g2x, iA, -1, AXA, g2y, iC, +1, AYC),
                (g3x, iB, -1, AXB, g3y, iD, -1, AYD),
            ]
            bs = []
            for c1, i1, s1, Aa1, c2, i2, s2, Aa2 in EDGES:
                bs.append((tt(V, c1, i1, Op.mult), tt(V, c2, i2, Op.mult)))
            nh = []
            for (c1, i1, s1, Aa1, c2, i2, s2, Aa2), (b1, b2_) in zip(EDGES, bs):
                n1 = gadd(b1, Aa1) if s1 > 0 else gsub(Aa1, b1)
                n2 = gadd(b2_, Aa2) if s2 > 0 else gsub(Aa2, b2_)
                h1 = tt(V, Aa1, b1, Op.subtract) if s1 > 0 else tt(V, Aa1, b1, Op.add)
                h2 = tt(V, Aa2, b2_, Op.subtract) if s2 > 0 else tt(V, Aa2, b2_, Op.add)
                nh.append((n1, h1, n2, h2))
                s.free(b1, b2_)
            dts = []
            for n1, h1, n2, h2 in nh:
                hi = tt(V, h1, h2, Op.min)
                nlo = tt(V, n1, n2, Op.min)
                s.free(n1, n2, h1, h2)
                hi1 = ts(V, hi, 1.0, Op.min)
                nlo0 = ts(V, nlo, 0.0, Op.min)
                s.free(hi, nlo)
                dte = tt(V, hi1, nlo0, Op.add)
                s.free(hi1, nlo0)
                dt = ts(V, dte, 0.0, Op.max)
                s.free(dte)
                dts.append(dt)
            dtP0, dtP1, dtP2, dtP3, dtT0, dtT1, dtT2, dtT3 = dts
            s.free(xi0, xi1, xi2, xi3, et0, et1, et2, et3,
                   g0x, g1x, g2x, g3x, g0y, g1y, g2y, g3y,
                   iu1, iu2, niw1, iw2, iA, iB, iC, iD,
                   AXU, AEU, AXW, AEW, AXA, AYC, AXB, AYD)

            # ---- Phase G: overlap, iou, loss -----------------------------------
            sp1 = tt(V, dtP0, dtP1, Op.add)
            sp2 = tt(V, dtP2, dtP3, Op.add)
            sp = tt(V, sp1, sp2, Op.add)
            ovP = tt(V, ab, sp, Op.mult)
            s.free(sp1, sp2, sp, ab, dtP0, dtP1, dtP2, dtP3)

            x0 = gsub(ab2, uu)
            x2 = gadd(ab2, uu)
            x1 = gadd(ab2, vv)
            x3 = gsub(ab2, vv)
            c0 = tt(V, dtT0, x0, Op.mult)
            c2_ = tt(V, dtT2, x2, Op.mult)
            c1_ = tt(V, dtT1, x1, Op.mult)
            c3_ = tt(V, dtT3, x3, Op.mult)
            s.free(x0, x1, x2, x3, uu, vv, ab2, dtT0, dtT1, dtT2, dtT3)
            cs1 = tt(V, c0, c2_, Op.add)
            cs2 = tt(V, c1_, c3_, Op.add)
            ovT = tt(V, cs1, cs2, Op.add)
            ov = tt(V, ovP, ovT, Op.add, dt=F32)
            s.free(c0, c1_, c2_, c3_, cs1, cs2, ovP, ovT)

            # log(iou) = max(ln(ov) - ln(den), ln(EPS))
            s12 = gadd(ar1, ar2, dt=F32)
            den = tt(V, s12, ov, Op.subtract, dt=F32)
            s.free(ar1, ar2, s12)
            ovc = ts(V, ov, 1e-35, Op.max, dt=F32)
            s.free(ov)
            lnd = act(den, Fn.Ln, bias=EPS, dt=F32)
            s.free(den)
            lno = act(ovc, Fn.Ln, dt=F32)
            s.free(ovc)
            df = tt(V, lno, lnd, Op.subtract, dt=F32)
            s.free(lno, lnd)

            acc = pool.tile([P, 1], F32, tag="acc")
            lg = ts(V, df, float(np.log(EPS)), Op.max, s2=0.0, op2=Op.add,
                    dt=F32, accum_out=acc[:, 0:1])
            s.free(df, lg)

            nc.sync.dma_start(out=out_h[:, :], in_=acc[:, :])

    nc.compile()
    return nc


_NC = None


def _get_nc():
    global _NC
    if _NC is None:
        _NC = _build()
    return _NC


class _Runner:
    """Cached PJRT executor for the compiled Bass module (see prior version)."""

    def __init__(self, nc):
        import jax
        from jax.sharding import Mesh, PartitionSpec
        try:
            from jax.experimental.shard_map import shard_map
        except ImportError:
            from jax.shard_map import shard_map
        from concourse import bass2jax, mybir as mb

        bass2jax.install_neuronx_cc_hook()
        self.jax = jax
        partition_name = (nc.partition_id_tensor.name
                          if nc.partition_id_tensor else None)
        in_names, out_names, out_avals, zero_outs = [], [], [], []
        for alloc in nc.m.functions[0].allocations:
            if not isinstance(alloc, mb.MemoryLocationSet):
                continue
            name = alloc.memorylocations[0].name
            if alloc.kind == "ExternalInput":
                if name != partition_name:
                    in_names.append(name)
            elif alloc.kind == "ExternalOutput":
                shape = tuple(alloc.tensor_shape)
                dtype = mb.dt.np(alloc.dtype)
                out_names.append(name)
                out_avals.append(jax.core.ShapedArray(shape, dtype))
                zero_outs.append(np.zeros((NCORES * shape[0],) + shape[1:], dtype))
        self.in_names = list(in_names)
        self.out_names = list(out_names)
        self.zero_outs = zero_outs
        n_params = len(in_names)
        all_names = in_names + out_names
        if partition_name is not None:
            all_names = all_names + [partition_name]

        def _body(*args):
            operands = list(args)
            if partition_name is not None:
                operands.append(bass2jax.partition_id_tensor())
            outs = bass2jax._bass_exec_p.bind(
                *operands,
                out_avals=tuple(out_avals),
                in_names=tuple(all_names),
                out_names=tuple(out_names),
                lowering_input_output_aliases=(),
                sim_require_finite=True,
                sim_require_nnan=True,
                nc=nc,
            )
            return tuple(outs)

        devices = jax.devices()[:NCORES]
        mesh = Mesh(np.asarray(devices), ("core",))
        n_outs = len(out_names)
        self.fn = jax.jit(
            shard_map(_body, mesh=mesh,
                      in_specs=(PartitionSpec("core"),) * (n_params + n_outs),
                      out_specs=(PartitionSpec("core"),) * n_outs,
                      check_rep=False),
            donate_argnums=tuple(range(n_params, n_params + n_outs)),
            keep_unused=True,
        )

    def __call__(self, pred, target):
        ins = {"pred": pred, "target": target}
        args = [ins[n] for n in self.in_names] + [z.copy() for z in self.zero_outs]
        outs = self.fn(*args)
        return [np.asarray(o) for o in outs]


_RUNNER = None


def _get_runner():
    global _RUNNER
    if _RUNNER is None:
        _RUNNER = _Runner(_get_nc())
    return _RUNNER


def kernel(pred: np.ndarray, target: np.ndarray) -> np.ndarray:
    pred = np.ascontiguousarray(np.asarray(pred, dtype=np.float32))
    target = np.ascontiguousarray(np.asarray(target, dtype=np.float32))
    assert pred.shape == (N_TOTAL, 5) and target.shape == (N_TOTAL, 5)

    runner = _get_runner()
    outs = runner(pred, target)
    total = outs[0].astype(np.float64).sum()
    return np.float32(-(total / N_TOTAL))
